# revision 16
# baseline (speedup 1.0000x reference)
"""Trainium2 Bass kernel for LittleBitLinear reconstruction.

Computes M = (sign(U_fp) * ell) @ sign(V_fp)^T * g[None, :] * h[:, None]
for U_fp (4096, 1024), V_fp (11008, 1024) -> M (4096, 11008) fp32.

Strategy: shard d_in (rows of V_fp / columns of M) across 8 cores; U_fp, h,
ell replicated. Each core computes the full 4096 x 1376 column block:
  - host passes U^T (1024, 4096) and the V^T shard (1024, 1376) so the
    contraction dim r lands on SBUF partitions (layout only, no math on host)
  - device computes A = bf16(sign(U^T) * ell) (lhsT) and
    B = bf16(sign(V^T) * g) (rhs); products are exact +-(ell*g) in bf16
    inputs with fp32 PSUM accumulation
  - 32 o-blocks x 3 n-tiles x 8 k-steps of 128x128x512 matmuls
  - PSUM evacuated via ScalarE activation copy fused with per-partition h
"""

import os
import sys

import numpy as np

for _p in ("/opt/trn_rl_repo",):
    if _p not in sys.path and os.path.isdir(_p):
        sys.path.insert(0, _p)

D_OUT, D_IN, R, NCORES = 4096, 11008, 1024, 8
N_SH = D_IN // NCORES  # 1376
P = 128


def _n_tiles(n_sh, max_n=512):
    tiles = []
    c0 = 0
    while c0 < n_sh:
        nw = min(max_n, n_sh - c0)
        tiles.append((c0, nw))
        c0 += nw
    return tiles


def build_program(d_out=D_OUT, n_sh=N_SH, r=R, reps=1, loop_n=None):
    """Build the per-core Bass program (SPMD: same program, different data).

    reps > 1 repeats the whole computation (for timing via slope); the
    output is simply rewritten each rep. loop_n wraps the body in a
    device-side For_i loop (timing: device time dominates dispatch).
    """
    from contextlib import ExitStack

    import concourse.bass as bass  # noqa: F401
    import concourse.mybir as mybir
    import concourse.tile as tile
    from concourse import bacc

    f32 = mybir.dt.float32
    bf16 = mybir.dt.bfloat16
    AF = mybir.ActivationFunctionType
    ALU = mybir.AluOpType

    kblocks = r // P          # 8
    oblocks = d_out // P      # 32
    OQ = 1024                 # o-columns per A staging chunk
    oq = min(OQ, d_out)
    nquarters = d_out // oq   # 4
    ntiles = _n_tiles(n_sh)   # [(0,512),(512,512),(1024,352)]

    nc = bacc.Bacc(None, target_bir_lowering=False)
    ut = nc.declare_dram_parameter("ut", [r, d_out], f32, isOutput=False)
    vt = nc.declare_dram_parameter("vt", [r, n_sh], f32, isOutput=False)
    ell = nc.declare_dram_parameter("ell", [P, kblocks], f32, isOutput=False)
    hh = nc.declare_dram_parameter("h", [P, oblocks], f32, isOutput=False)
    gg = nc.declare_dram_parameter("g", [P, n_sh], f32, isOutput=False)
    out = nc.declare_dram_parameter("out", [d_out, n_sh], f32, isOutput=True)

    with tile.TileContext(nc) as tc, ExitStack() as ctx:
        consts = ctx.enter_context(tc.tile_pool(name="consts", bufs=1))
        apool = ctx.enter_context(tc.tile_pool(name="apool", bufs=kblocks * nquarters))
        bpool = ctx.enter_context(tc.tile_pool(name="bpool", bufs=kblocks * len(ntiles)))
        ustg = ctx.enter_context(tc.tile_pool(name="ustg", bufs=4))
        vstg = ctx.enter_context(tc.tile_pool(name="vstg", bufs=4))
        usgn = ctx.enter_context(tc.tile_pool(name="usgn", bufs=4))
        vsgn = ctx.enter_context(tc.tile_pool(name="vsgn", bufs=4))
        outp = ctx.enter_context(tc.tile_pool(name="outp", bufs=3))
        psum = ctx.enter_context(tc.tile_pool(name="psum", bufs=6, space="PSUM"))

        # Route every shared operand through one ACT copy so downstream DVE
        # ops (TT/TS) carry a single cross-proc wait (walrus S3S3D3 TT
        # struct holds only one sync-wait slot).
        ell_raw = consts.tile([P, kblocks], f32)
        nc.sync.dma_start(out=ell_raw, in_=ell[:, :])
        ell_sb = consts.tile([P, kblocks], f32)
        nc.scalar.activation(out=ell_sb, in_=ell_raw, func=AF.Copy)
        h_raw = consts.tile([P, oblocks], f32)
        nc.sync.dma_start(out=h_raw, in_=hh[:, :])
        h_sb = consts.tile([P, oblocks], f32)
        nc.scalar.activation(out=h_sb, in_=h_raw, func=AF.Copy)

        # g arrives host-replicated across partitions; downcast to bf16 once
        g_f32 = consts.tile([P, n_sh], f32)
        nc.sync.dma_start(out=g_f32, in_=gg[:, :])
        g_bc = consts.tile([P, n_sh], bf16)
        nc.scalar.activation(out=g_bc, in_=g_f32, func=AF.Copy)

        # --- B = bf16(sign(V^T) * g), tiled (k, n); A = bf16(sign(U^T) * ell)
        # tiled (q, k). Interleave V and first-quarter U loads so the first
        # o-block's accumulation chain is fed as early as possible.
        loop_cm = (
            tc.For_i(0, loop_n, 1, hint_engines=(mybir.EngineType.PE,))
            if loop_n is not None
            else None
        )
        if loop_cm is not None:
            ctx.enter_context(loop_cm)
        for rep in range(reps):
            btiles = {}
            atiles = {}

            def stage_b(k):
                for n, (c0, nw) in enumerate(ntiles):
                    vst = vstg.tile([P, nw], f32, tag="vstg", name=f"vst_{rep}_{k}_{n}")
                    nc.sync.dma_start(out=vst, in_=vt[k * P:(k + 1) * P, c0:c0 + nw])
                    vs = vsgn.tile([P, nw], bf16, tag="vsgn", name=f"vs_{rep}_{k}_{n}")
                    nc.scalar.activation(out=vs, in_=vst, func=AF.Sign)
                    bt = bpool.tile([P, nw], bf16, tag="b", name=f"b_{rep}_{k}_{n}")
                    nc.vector.tensor_tensor(
                        out=bt, in0=vs, in1=g_bc[:, c0:c0 + nw], op=ALU.mult
                    )
                    btiles[(k, n)] = bt

            def stage_a(q, k):
                ust = ustg.tile([P, oq], f32, tag="ustg", name=f"ust_{rep}_{q}_{k}")
                nc.sync.dma_start(
                    out=ust, in_=ut[k * P:(k + 1) * P, q * oq:(q + 1) * oq]
                )
                us = usgn.tile([P, oq], bf16, tag="usgn", name=f"us_{rep}_{q}_{k}")
                nc.scalar.activation(out=us, in_=ust, func=AF.Sign)
                at = apool.tile([P, oq], bf16, tag="a", name=f"a_{rep}_{q}_{k}")
                nc.vector.tensor_scalar(
                    out=at, in0=us, scalar1=ell_sb[:, k:k + 1], scalar2=None,
                    op0=ALU.mult,
                )
                atiles[(q, k)] = at

            for k in range(kblocks):
                stage_b(k)
                stage_a(0, k)
            for q in range(1, nquarters):
                for k in range(kblocks):
                    stage_a(q, k)

            # --- matmul + evacuate
            obl_per_q = oq // P
            for j in range(oblocks):
                q, jq = divmod(j, obl_per_q)
                col = jq * P
                ot = outp.tile([P, n_sh], f32, tag="out", name=f"ot_{rep}_{j}")
                pts = [
                    psum.tile([P, nw], f32, tag="ps", name=f"ps_{rep}_{j}_{ni}")
                    for ni, (c0, nw) in enumerate(ntiles)
                ]
                for k in range(kblocks):
                    lhsT = atiles[(q, k)][:, col:col + P]
                    for n, (c0, nw) in enumerate(ntiles):
                        nc.tensor.matmul(
                            pts[n], lhsT=lhsT, rhs=btiles[(k, n)],
                            start=(k == 0), stop=(k == kblocks - 1),
                        )
                for n, (c0, nw) in enumerate(ntiles):
                    nc.scalar.activation(
                        out=ot[:, c0:c0 + nw], in_=pts[n], func=AF.Copy,
                        scale=h_sb[:, j:j + 1],
                    )
                nc.sync.dma_start(out=out[j * P:(j + 1) * P, :], in_=ot)

    nc.compile()
    return nc


_NC_CACHE = {}


def _get_nc():
    if "nc" not in _NC_CACHE:
        _NC_CACHE["nc"] = build_program()
    return _NC_CACHE["nc"]


def _make_in_maps(U_fp, V_fp, h, g, ell):
    U_fp = np.ascontiguousarray(np.asarray(U_fp, dtype=np.float32))
    V_fp = np.ascontiguousarray(np.asarray(V_fp, dtype=np.float32))
    h = np.asarray(h, dtype=np.float32).reshape(-1)
    g = np.asarray(g, dtype=np.float32).reshape(-1)
    ell = np.asarray(ell, dtype=np.float32).reshape(-1)

    ut = np.ascontiguousarray(U_fp.T)                      # (R, D_OUT)
    ell_t = np.ascontiguousarray(ell.reshape(R // P, P).T)  # (128, 8)
    h_t = np.ascontiguousarray(h.reshape(D_OUT // P, P).T)  # (128, 32)

    in_maps = []
    for c in range(NCORES):
        sl = slice(c * N_SH, (c + 1) * N_SH)
        in_maps.append({
            "ut": ut,
            "vt": np.ascontiguousarray(V_fp[sl, :].T),     # (R, N_SH)
            "ell": ell_t,
            "h": h_t,
            "g": np.ascontiguousarray(
                np.broadcast_to(g[sl].reshape(1, N_SH), (P, N_SH))
            ),
        })
    return in_maps


def run(U_fp, V_fp, h, g, ell, trace=False):
    """Run on 8 NeuronCores; returns (M, BassKernelResults)."""
    from concourse.bass_utils import run_bass_kernel_spmd

    nc = _get_nc()
    in_maps = _make_in_maps(U_fp, V_fp, h, g, ell)
    res = run_bass_kernel_spmd(nc, in_maps, list(range(NCORES)), trace=trace)
    M = np.concatenate([res.results[c]["out"] for c in range(NCORES)], axis=1)
    return M, res


def kernel(U_fp, V_fp, h, g, ell):
    M, _ = run(U_fp, V_fp, h, g, ell, trace=False)
    return M


# revision 21
# speedup vs baseline: 1.0868x; 1.0868x over previous
"""Trainium2 Bass kernel for LittleBitLinear reconstruction.

Computes M = (sign(U_fp) * ell) @ sign(V_fp)^T * g[None, :] * h[:, None]
for U_fp (4096, 1024), V_fp (11008, 1024) -> M (4096, 11008) fp32.

Strategy: shard d_in (rows of V_fp / columns of M) across 8 cores; U_fp, h,
ell replicated. Each core computes the full 4096 x 1376 column block:
  - host passes U^T (1024, 4096) and the V^T shard (1024, 1376) so the
    contraction dim r lands on SBUF partitions (layout only, no math on host)
  - device computes A = bf16(sign(U^T) * ell) (lhsT) and
    B = bf16(sign(V^T) * g) (rhs); products are exact +-(ell*g) in bf16
    inputs with fp32 PSUM accumulation
  - 32 o-blocks x 3 n-tiles x 8 k-steps of 128x128x512 matmuls
  - PSUM evacuated via ScalarE activation copy fused with per-partition h
"""

import os
import sys

import numpy as np

for _p in ("/opt/trn_rl_repo",):
    if _p not in sys.path and os.path.isdir(_p):
        sys.path.insert(0, _p)

D_OUT, D_IN, R, NCORES = 4096, 11008, 1024, 8
N_SH = D_IN // NCORES  # 1376
P = 128


def _n_tiles(n_sh, max_n=512):
    tiles = []
    c0 = 0
    while c0 < n_sh:
        nw = min(max_n, n_sh - c0)
        tiles.append((c0, nw))
        c0 += nw
    return tiles


def build_program(d_out=D_OUT, n_sh=N_SH, r=R, reps=1, loop_n=None, skip=()):
    """Build the per-core Bass program (SPMD: same program, different data).

    reps > 1 repeats the whole computation (for timing via slope); the
    output is simply rewritten each rep. loop_n wraps the body in a
    device-side For_i loop (timing: device time dominates dispatch).
    """
    from contextlib import ExitStack

    import concourse.bass as bass  # noqa: F401
    import concourse.mybir as mybir
    import concourse.tile as tile
    from concourse import bacc

    f32 = mybir.dt.float32
    bf16 = mybir.dt.bfloat16
    AF = mybir.ActivationFunctionType
    ALU = mybir.AluOpType

    kblocks = r // P          # 8
    oblocks = d_out // P      # 32
    OQ = 1024                 # o-columns per A staging chunk
    oq = min(OQ, d_out)
    nquarters = d_out // oq   # 4
    ntiles = _n_tiles(n_sh)   # [(0,512),(512,512),(1024,352)]

    nc = bacc.Bacc(None, target_bir_lowering=False)
    ut = nc.declare_dram_parameter("ut", [r, d_out], f32, isOutput=False)
    vt = nc.declare_dram_parameter("vt", [r, n_sh], f32, isOutput=False)
    ell = nc.declare_dram_parameter("ell", [P, kblocks], f32, isOutput=False)
    hh = nc.declare_dram_parameter("h", [P, oblocks], f32, isOutput=False)
    gg = nc.declare_dram_parameter("g", [P, n_sh], f32, isOutput=False)
    out = nc.declare_dram_parameter("out", [d_out, n_sh], f32, isOutput=True)

    with tile.TileContext(nc) as tc, ExitStack() as ctx:
        consts = ctx.enter_context(tc.tile_pool(name="consts", bufs=1))
        apool = ctx.enter_context(tc.tile_pool(name="apool", bufs=kblocks * nquarters))
        bpool = ctx.enter_context(tc.tile_pool(name="bpool", bufs=kblocks * len(ntiles)))
        ustg = ctx.enter_context(tc.tile_pool(name="ustg", bufs=6))
        vstg = ctx.enter_context(tc.tile_pool(name="vstg", bufs=6))
        usgn = ctx.enter_context(tc.tile_pool(name="usgn", bufs=4))
        vsgn = ctx.enter_context(tc.tile_pool(name="vsgn", bufs=4))
        outp = ctx.enter_context(tc.tile_pool(name="outp", bufs=4))
        psum = ctx.enter_context(tc.tile_pool(name="psum", bufs=8, space="PSUM"))

        # Route every shared operand through one ACT copy so downstream DVE
        # ops (TT/TS) carry a single cross-proc wait (walrus S3S3D3 TT
        # struct holds only one sync-wait slot).
        ell_raw = consts.tile([P, kblocks], f32)
        nc.sync.dma_start(out=ell_raw, in_=ell[:, :])
        ell_sb = consts.tile([P, kblocks], f32)
        nc.scalar.activation(out=ell_sb, in_=ell_raw, func=AF.Copy)
        h_raw = consts.tile([P, oblocks], f32)
        nc.sync.dma_start(out=h_raw, in_=hh[:, :])
        h_sb = consts.tile([P, oblocks], f32)
        nc.scalar.activation(out=h_sb, in_=h_raw, func=AF.Copy)

        # g arrives host-replicated across partitions; downcast to bf16 once
        g_f32 = consts.tile([P, n_sh], f32)
        nc.sync.dma_start(out=g_f32, in_=gg[:, :])
        g_bc = consts.tile([P, n_sh], bf16)
        nc.scalar.activation(out=g_bc, in_=g_f32, func=AF.Copy)

        # --- B = bf16(sign(V^T) * g), tiled (k, n); A = bf16(sign(U^T) * ell)
        # tiled (q, k). Interleave V and first-quarter U loads so the first
        # o-block's accumulation chain is fed as early as possible.
        loop_cm = (
            tc.For_i(0, loop_n, 1, hint_engines=(mybir.EngineType.PE,))
            if loop_n is not None
            else None
        )
        if loop_cm is not None:
            ctx.enter_context(loop_cm)
        for rep in range(reps):
            btiles = {}
            atiles = {}

            def stage_b(k):
                for n, (c0, nw) in enumerate(ntiles):
                    vst = vstg.tile([P, nw], f32, tag="vstg", name=f"vst_{rep}_{k}_{n}")
                    nc.sync.dma_start(out=vst, in_=vt[k * P:(k + 1) * P, c0:c0 + nw])
                    vs = vsgn.tile([P, nw], bf16, tag="vsgn", name=f"vs_{rep}_{k}_{n}")
                    nc.scalar.activation(out=vs, in_=vst, func=AF.Sign)
                    bt = bpool.tile([P, nw], bf16, tag="b", name=f"b_{rep}_{k}_{n}")
                    nc.vector.tensor_tensor(
                        out=bt, in0=vs, in1=g_bc[:, c0:c0 + nw], op=ALU.mult
                    )
                    btiles[(k, n)] = bt

            def stage_a(q, k):
                ust = ustg.tile([P, oq], f32, tag="ustg", name=f"ust_{rep}_{q}_{k}")
                nc.sync.dma_start(
                    out=ust, in_=ut[k * P:(k + 1) * P, q * oq:(q + 1) * oq]
                )
                us = usgn.tile([P, oq], bf16, tag="usgn", name=f"us_{rep}_{q}_{k}")
                nc.scalar.activation(out=us, in_=ust, func=AF.Sign)
                at = apool.tile([P, oq], bf16, tag="a", name=f"a_{rep}_{q}_{k}")
                nc.vector.tensor_scalar(
                    out=at, in0=us, scalar1=ell_sb[:, k:k + 1], scalar2=None,
                    op0=ALU.mult,
                )
                atiles[(q, k)] = at

            if "stage" not in skip:
                for k in range(kblocks):
                    stage_b(k)
                    stage_a(0, k)
                for q in range(1, nquarters):
                    for k in range(kblocks):
                        stage_a(q, k)
            else:
                for k in range(kblocks):
                    for n, (c0, nw) in enumerate(ntiles):
                        btiles[(k, n)] = bpool.tile(
                            [P, nw], bf16, tag="b", name=f"b_{rep}_{k}_{n}"
                        )
                for q in range(nquarters):
                    for k in range(kblocks):
                        atiles[(q, k)] = apool.tile(
                            [P, oq], bf16, tag="a", name=f"a_{rep}_{q}_{k}"
                        )

            # --- matmul + evacuate
            obl_per_q = oq // P
            for j in range(oblocks):
                q, jq = divmod(j, obl_per_q)
                col = jq * P
                ot = outp.tile([P, n_sh], f32, tag="out", name=f"ot_{rep}_{j}")
                pts = [
                    psum.tile([P, nw], f32, tag="ps", name=f"ps_{rep}_{j}_{ni}")
                    for ni, (c0, nw) in enumerate(ntiles)
                ]
                if "mm" not in skip:
                    for k in range(kblocks):
                        lhsT = atiles[(q, k)][:, col:col + P]
                        for n, (c0, nw) in enumerate(ntiles):
                            nc.tensor.matmul(
                                pts[n], lhsT=lhsT, rhs=btiles[(k, n)],
                                start=(k == 0), stop=(k == kblocks - 1),
                            )
                if "evac" not in skip:
                    for n, (c0, nw) in enumerate(ntiles):
                        nc.scalar.activation(
                            out=ot[:, c0:c0 + nw], in_=pts[n], func=AF.Copy,
                            scale=h_sb[:, j:j + 1],
                        )
                else:
                    nc.vector.memset(ot[:, 0:1], 0.0)
                if "outdma" not in skip:
                    # ACT's HWDGE queue: keeps stores off the input-load queue
                    nc.scalar.dma_start(out=out[j * P:(j + 1) * P, :], in_=ot)

    nc.compile()
    return nc


_NC_CACHE = {}


def _get_nc():
    if "nc" not in _NC_CACHE:
        _NC_CACHE["nc"] = build_program()
    return _NC_CACHE["nc"]


def _make_in_maps(U_fp, V_fp, h, g, ell):
    U_fp = np.ascontiguousarray(np.asarray(U_fp, dtype=np.float32))
    V_fp = np.ascontiguousarray(np.asarray(V_fp, dtype=np.float32))
    h = np.asarray(h, dtype=np.float32).reshape(-1)
    g = np.asarray(g, dtype=np.float32).reshape(-1)
    ell = np.asarray(ell, dtype=np.float32).reshape(-1)

    ut = np.ascontiguousarray(U_fp.T)                      # (R, D_OUT)
    ell_t = np.ascontiguousarray(ell.reshape(R // P, P).T)  # (128, 8)
    h_t = np.ascontiguousarray(h.reshape(D_OUT // P, P).T)  # (128, 32)

    in_maps = []
    for c in range(NCORES):
        sl = slice(c * N_SH, (c + 1) * N_SH)
        in_maps.append({
            "ut": ut,
            "vt": np.ascontiguousarray(V_fp[sl, :].T),     # (R, N_SH)
            "ell": ell_t,
            "h": h_t,
            "g": np.ascontiguousarray(
                np.broadcast_to(g[sl].reshape(1, N_SH), (P, N_SH))
            ),
        })
    return in_maps


def run(U_fp, V_fp, h, g, ell, trace=False):
    """Run on 8 NeuronCores; returns (M, BassKernelResults)."""
    from concourse.bass_utils import run_bass_kernel_spmd

    nc = _get_nc()
    in_maps = _make_in_maps(U_fp, V_fp, h, g, ell)
    res = run_bass_kernel_spmd(nc, in_maps, list(range(NCORES)), trace=trace)
    M = np.concatenate([res.results[c]["out"] for c in range(NCORES)], axis=1)
    return M, res


def kernel(U_fp, V_fp, h, g, ell):
    M, _ = run(U_fp, V_fp, h, g, ell, trace=False)
    return M
